# revision 27
# baseline (speedup 1.0000x reference)
"""ContactPointMamba Trainium2 kernel (8 NeuronCores).

Sharding: core = batch(2) x seq-chunk(4). The model serializes the point
cloud into a 2048-token sequence (forward-order 1024 ++ backward-order
1024); each core owns 512 contiguous tokens of one batch's sequence and
the full d_inner. Per layer, two small AllGathers within each 4-core
group move (a) the 3-token depthwise-conv halo and (b) the scan-carry
summaries (chunk decay P and local final state) across chunks.

Host does only index math (Morton codes, stable argsorts), gathers /
concat / transposes, and the final un-gather. All float math runs on
device. Residual stream is kept transposed [384, L]; matmuls run fp32r;
the Mamba recurrence uses the DVE/GPSIMD tensor_tensor_scan; the
cross-chunk carry is applied with a cumprod fixup (CP = cumprod(dA),
h_full = h_loc + CP*h_in), then y = sum_n C_n*h_n + D*u, gated by
silu(z).
"""

import os
import numpy as np

import concourse.bass as bass
import concourse.tile as tile
import concourse.mybir as mybir
from concourse import bacc
from concourse.bass_utils import run_bass_kernel_spmd

# model dims
BS, N, D = 2, 1024, 384
DEPTH, DI, DS, DR, K = 12, 768, 16, 24, 4
CONTACT, PF, TF, TE, LAST = 3, 512, 512, 512, 3
INCH = CONTACT + TF + TE + PF + 3          # 1542
INCH_PAD = 13 * 128                        # 1664
L = 2 * N                                  # 2048
NCORES = 8
NCHUNK = 4
LC = L // NCHUNK                           # 512
FT = D // 128                              # 3 feature tiles
DT6 = DI // 128                            # 6 d_inner tiles
GROUPS = [[0, 1, 2, 3], [4, 5, 6, 7]]
NGRPS = [list(range(0, 6)), list(range(6, 12)), list(range(12, 16))]

f32 = mybir.dt.float32
f32r = mybir.dt.float32r
bf16 = mybir.dt.bfloat16
AF = mybir.ActivationFunctionType
OP = mybir.AluOpType

# VDI pack columns: conv_b, dt_b, D_p, conv_w[4], A_log[16]
VDI_CB, VDI_DTB, VDI_DP, VDI_CW, VDI_AL = 0, 1, 2, 3, 7
VDI_W = 23
# EV2 pack columns (per ftile): emb_b2, ln2_w, ln2_b, pos_b2, normf_w, normf_b
EV2_B2, EV2_LW, EV2_LB, EV2_PB2, EV2_NFW, EV2_NFB = 0, 1, 2, 3, 4, 5
EV2_W = 6

_CACHE = {}


def _morton_np(xyz, perm):
    """numpy replica of the reference _morton (float32 ops, bit-exact)."""
    xyz = xyz.astype(np.float32)
    mn = xyz.min(axis=1, keepdims=True)
    mx = xyz.max(axis=1, keepdims=True)
    g = (xyz - mn) / (mx - mn + np.float32(1e-6)) * np.float32(1023.0)
    g = np.clip(g, np.float32(0.0), np.float32(1023.0)).astype(np.int32)
    g = g[..., list(perm)]
    code = np.zeros(g.shape[:2], np.int32)
    for bit in range(10):
        for ax in range(3):
            code = code | (((g[..., ax] >> bit) & 1) << (3 * bit + ax))
    return code


def _transposed_ln(nc, pools, x_tiles_f32r, gamma_cols, beta_cols, out_dtype=f32r):
    """LayerNorm across the partition (feature) axis of FT tiles [128, LC].

    x_tiles_f32r: list of FT APs [128, LC] tagged f32r (matmul rhs).
    gamma_cols/beta_cols: list of FT per-partition scalar APs [128, 1].
    Returns list of FT APs [128, LC] (out_dtype-tagged) from the wk pool.
    """
    scr, psA, wk = pools["scr"], pools["psA"], pools["wk"]
    nft = len(x_tiles_f32r)
    ones_r = pools["ones_r"]
    ps_s_t = psA.tile([56, LC], f32, tag="sm")
    ps_s = ps_s_t[0:1, :]
    ps_ss_t = psA.tile([56, LC], f32, tag="sm")
    ps_ss = ps_ss_t[0:1, :]
    for kt in range(nft):
        nc.tensor.matmul(out=ps_s, lhsT=ones_r[:, 0:1], rhs=x_tiles_f32r[kt],
                         start=(kt == 0), stop=(kt == nft - 1))
    sqt = scr.tile([128, LC], f32r, tag="s")
    for kt in range(nft):
        nc.scalar.activation(out=sqt, in_=x_tiles_f32r[kt], func=AF.Square)
        nc.tensor.matmul(out=ps_ss, lhsT=ones_r[:, 0:1], rhs=sqt,
                         start=(kt == 0), stop=(kt == nft - 1))
    inv_n = 1.0 / (128 * nft)
    mu = scr.tile([1, LC], f32, tag="s")
    ms = scr.tile([1, LC], f32, tag="s")
    nc.scalar.mul(out=mu, in_=ps_s, mul=inv_n)
    nc.scalar.mul(out=ms, in_=ps_ss, mul=inv_n)
    mu2 = scr.tile([1, LC], f32, tag="s")
    nc.vector.tensor_mul(mu2, mu, mu)
    var = scr.tile([1, LC], f32, tag="s")
    nc.vector.tensor_sub(var, ms, mu2)
    # rstd = exp(-0.5 * ln(var + eps)); avoids sqrt (stays on exp/ln table)
    lnv = scr.tile([1, LC], f32, tag="s")
    nc.scalar.activation(out=lnv, in_=var, func=AF.Ln, bias=pools["eps1"][0:1, 0:1])
    rstd = scr.tile([1, LC], f32, tag="s")
    nc.scalar.activation(out=rstd, in_=lnv, func=AF.Exp, scale=-0.5)
    mub = scr.tile([128, LC], f32, tag="s")
    rb = scr.tile([128, LC], f32, tag="s")
    nc.gpsimd.partition_broadcast(out_ap=mub, in_ap=mu)
    nc.gpsimd.partition_broadcast(out_ap=rb, in_ap=rstd)
    outs = []
    for kt in range(nft):
        t1 = scr.tile([128, LC], f32, tag="s")
        nc.vector.tensor_sub(t1, x_tiles_f32r[kt].bitcast(f32), mub)
        t2 = scr.tile([128, LC], f32, tag="s")
        nc.vector.tensor_mul(t2, t1, rb)
        o = wk.tile([128, LC], out_dtype, tag=f"ln_o{kt}")
        nc.vector.tensor_scalar(out=o, in0=t2, scalar1=gamma_cols[kt],
                                scalar2=beta_cols[kt], op0=OP.mult, op1=OP.add)
        outs.append(o)
    return outs


def _build_nc():
    nc = bacc.Bacc("TRN2", target_bir_lowering=False, debug=False,
                   num_devices=NCORES)

    # ---- I/O -------------------------------------------------------------
    din = {}

    def inp(name, shape, dtype=f32):
        din[name] = nc.dram_tensor(name, shape, dtype, kind="ExternalInput")
        return din[name]

    inp("fusionT", [128, 13, LC])
    inp("xyzT", [3, LC])
    inp("EW1", [128, 13, 128])
    inp("EV1", [128, 3])
    inp("EW2", [128, FT, 128])
    inp("PW1", [3, 128])
    inp("PB1", [128, 1])
    inp("PW2", [128, FT, 128])
    inp("EV2", [128, FT, EV2_W])
    inp("OSV", [128, FT, 2])
    inp("W_in", [128, DEPTH, FT, 12, 128])
    inp("W_out", [128, DEPTH, DT6, FT, 128])
    inp("W_xp", [128, DEPTH, DT6, 56])
    inp("W_dt", [24, DEPTH, DT6, 128])
    inp("VD", [128, DEPTH, FT, 2])
    inp("VDI", [128, DEPTH, DT6, VDI_W])
    inp("OW", [128, FT, LAST])
    inp("OB", [LAST, 1])
    inp("ONESC", [128, 1])
    inp("ONESR", [1, 128])
    inp("selA", [128, NCHUNK])
    inp("selB", [128, NCHUNK - 1])
    out_t = nc.dram_tensor("out", [LAST, LC], f32, kind="ExternalOutput")

    from contextlib import ExitStack
    with tile.TileContext(nc) as tc, ExitStack() as ctx, \
            nc.allow_low_precision(reason="f32r tags carry full fp32 bits"):
        cst = ctx.enter_context(tc.tile_pool(name="cst", bufs=1))
        statep = ctx.enter_context(tc.tile_pool(name="state", bufs=1))
        wk = ctx.enter_context(tc.tile_pool(name="wk", bufs=1))
        prm = ctx.enter_context(tc.tile_pool(name="prm", bufs=1))
        scr = ctx.enter_context(tc.tile_pool(name="scr", bufs=8))
        psA = ctx.enter_context(tc.tile_pool(name="psA", bufs=2, space="PSUM"))
        psB = ctx.enter_context(tc.tile_pool(name="psB", bufs=4, space="PSUM"))
        dram = ctx.enter_context(tc.tile_pool(name="dram", bufs=1, space="DRAM"))

        # constants
        ones_r = cst.tile([128, 1], f32r)
        nc.sync.dma_start(out=ones_r, in_=din["ONESC"][:, :].bitcast(f32r))
        ones_f = cst.tile([128, 1], f32)
        nc.sync.dma_start(out=ones_f, in_=din["ONESC"][:, :])
        ones_row = cst.tile([1, 128], f32r)
        nc.sync.dma_start(out=ones_row, in_=din["ONESR"][:, :].bitcast(f32r))
        eps1 = cst.tile([128, 1], f32)
        nc.vector.memset(eps1, 1e-5)
        zeros512 = cst.tile([128, LC], f32)
        nc.vector.memset(zeros512, 0.0)
        selA_t = cst.tile([128, NCHUNK], f32)
        nc.sync.dma_start(out=selA_t, in_=din["selA"][:, :])
        selB_t = cst.tile([128, NCHUNK - 1], f32)
        nc.sync.dma_start(out=selB_t, in_=din["selB"][:, :])
        osv = cst.tile([128, FT, 2], f32)
        nc.sync.dma_start(out=osv, in_=din["OSV"][:, :, :])
        ev2 = cst.tile([128, FT, EV2_W], f32)
        nc.sync.dma_start(out=ev2, in_=din["EV2"][:, :, :])
        pools = {"scr": scr, "psA": psA, "wk": wk, "ones_r": ones_r,
                 "eps1": eps1}

        hid = statep.tile([128, FT, LC], f32r, tag="hid")

        # ---- embedding ---------------------------------------------------
        with tc.tile_pool(name="emb", bufs=1) as emb:
            ev1 = emb.tile([128, 3], f32)
            nc.sync.dma_start(out=ev1, in_=din["EV1"][:, :])
            ew2 = emb.tile([128, FT, 128], f32r)
            nc.sync.dma_start(out=ew2, in_=din["EW2"][:, :, :].bitcast(f32r))
            xyz3 = emb.tile([3, LC], f32r)
            nc.sync.dma_start(out=xyz3, in_=din["xyzT"][:, :].bitcast(f32r))
            pw1 = emb.tile([3, 128], f32r)
            nc.sync.dma_start(out=pw1, in_=din["PW1"][:, :].bitcast(f32r))
            pb1 = emb.tile([128, 1], f32)
            nc.sync.dma_start(out=pb1, in_=din["PB1"][:, :])
            pw2 = emb.tile([128, FT, 128], f32r)
            nc.sync.dma_start(out=pw2, in_=din["PW2"][:, :, :].bitcast(f32r))

            ps1 = psB.tile([128, LC], f32, tag="mm")
            for kt in range(13):
                ftk = emb.tile([128, LC], f32r, tag="ftk")
                nc.sync.dma_start(out=ftk, in_=din["fusionT"][:, kt].bitcast(f32r))
                ew1k = emb.tile([128, 128], f32r, tag="ew1k")
                nc.sync.dma_start(out=ew1k, in_=din["EW1"][:, kt].bitcast(f32r))
                nc.tensor.matmul(out=ps1, lhsT=ew1k, rhs=ftk,
                                 start=(kt == 0), stop=(kt == 12))
            e1 = emb.tile([128, LC], f32r)
            nc.scalar.activation(out=e1, in_=ps1, func=AF.Identity,
                                 bias=ev1[:, 0:1])
            xn1 = _transposed_ln(nc, pools, [e1], [ev1[:, 1:2]], [ev1[:, 2:3]],
                                 out_dtype=f32)
            g1 = emb.tile([128, LC], f32r)
            nc.scalar.activation(out=g1, in_=xn1[0], func=AF.Gelu_apprx_tanh)

            h2 = []
            for mt in range(FT):
                ps2 = psB.tile([128, LC], f32, tag="mm")
                nc.tensor.matmul(out=ps2, lhsT=ew2[:, mt, :], rhs=g1,
                                 start=True, stop=True)
                e2 = emb.tile([128, LC], f32r, tag=f"e2_{mt}")
                nc.scalar.activation(out=e2, in_=ps2, func=AF.Identity,
                                     bias=ev2[:, mt, EV2_B2:EV2_B2 + 1])
                h2.append(e2)
            xn2 = _transposed_ln(
                nc, pools, h2,
                [ev2[:, mt, EV2_LW:EV2_LW + 1] for mt in range(FT)],
                [ev2[:, mt, EV2_LB:EV2_LB + 1] for mt in range(FT)],
                out_dtype=f32)

            # pos MLP
            psp = psB.tile([128, LC], f32, tag="mm")
            nc.tensor.matmul(out=psp, lhsT=pw1, rhs=xyz3, start=True, stop=True)
            p1 = emb.tile([128, LC], f32)
            nc.scalar.activation(out=p1, in_=psp, func=AF.Identity, bias=pb1[:, 0:1])
            pg = emb.tile([128, LC], f32r)
            nc.scalar.activation(out=pg, in_=p1, func=AF.Gelu_apprx_tanh)
            for mt in range(FT):
                psq = psB.tile([128, LC], f32, tag="mm")
                nc.tensor.matmul(out=psq, lhsT=pw2[:, mt, :], rhs=pg,
                                 start=True, stop=True)
                pos = emb.tile([128, LC], f32, tag=f"pos_{mt}")
                nc.scalar.activation(out=pos, in_=psq, func=AF.Identity,
                                     bias=ev2[:, mt, EV2_PB2:EV2_PB2 + 1])
                t = emb.tile([128, LC], f32, tag=f"osx_{mt}")
                nc.vector.tensor_scalar(out=t, in0=xn2[mt],
                                        scalar1=osv[:, mt, 0:1],
                                        scalar2=osv[:, mt, 1:2],
                                        op0=OP.mult, op1=OP.add)
                nc.vector.tensor_add(hid[:, mt, :], t, pos)

        # ---- layers ------------------------------------------------------
        hbp = ctx.enter_context(tc.tile_pool(name="hbp", bufs=1))
        cpp = ctx.enter_context(tc.tile_pool(name="cpp", bufs=1))
        bcp = ctx.enter_context(tc.tile_pool(name="bcp", bufs=1))
        sml = ctx.enter_context(tc.tile_pool(name="sml", bufs=3))

        xs = wk.tile([128, DT6, 3 + LC], f32, tag="xs")
        zs = wk.tile([128, DT6, LC], bf16, tag="zs")
        u = wk.tile([128, DT6, LC], f32r, tag="u")
        dtt = wk.tile([128, DT6, LC], f32, tag="dtt")
        dtu = wk.tile([128, DT6, LC], bf16, tag="dtu")
        dbl = wk.tile([56, LC], f32r, tag="dbl")
        yt = wk.tile([128, DT6, LC], f32r, tag="yt")


        for li in range(int(os.environ.get("KERNEL_DEPTH", str(DEPTH)))):
            w_xp = prm.tile([128, DT6, 56], f32r, tag="w_xp")
            nc.sync.dma_start(out=w_xp, in_=din["W_xp"][:, li].bitcast(f32r))
            w_dt = prm.tile([24, DT6, 128], f32r, tag="w_dt")
            nc.sync.dma_start(out=w_dt, in_=din["W_dt"][:, li].bitcast(f32r))
            vd = prm.tile([128, FT, 2], f32, tag="vd")
            nc.sync.dma_start(out=vd, in_=din["VD"][:, li])
            vdi = prm.tile([128, DT6, VDI_W], f32, tag="vdi")
            nc.sync.dma_start(out=vdi, in_=din["VDI"][:, li])
            aneg = prm.tile([128, DT6, DS], f32, tag="aneg")
            nc.scalar.activation(out=aneg, in_=vdi[:, :, VDI_AL:VDI_AL + DS],
                                 func=AF.Exp)
            nc.scalar.mul(out=aneg, in_=aneg, mul=-1.0)

            xln = _transposed_ln(
                nc, pools, [hid[:, mt, :] for mt in range(FT)],
                [vd[:, mt, 0:1] for mt in range(FT)],
                [vd[:, mt, 1:2] for mt in range(FT)], out_dtype=f32r)

            # in_proj: m-tiles 0..5 -> xs, 6..11 -> z
            for mt in range(12):
                w_in = prm.tile([128, FT, 128], f32r, tag="w_in")
                nc.sync.dma_start(out=w_in,
                                  in_=din["W_in"][:, li, :, mt, :].bitcast(f32r))
                ps = psB.tile([128, LC], f32, tag="mm")
                for kt in range(FT):
                    nc.tensor.matmul(out=ps, lhsT=w_in[:, kt, :],
                                     rhs=xln[kt], start=(kt == 0),
                                     stop=(kt == FT - 1))
                if mt < DT6:
                    nc.scalar.copy(out=xs[:, mt, 3:3 + LC], in_=ps)
                else:
                    nc.scalar.copy(out=zs[:, mt - DT6, :], in_=ps)

            # exchange A: conv halo
            ccA_in = dram.tile([128, DT6, 3], f32, tag="ccA_in")
            ccA_out = dram.tile([NCHUNK, 128, DT6, 3], f32, tag="ccA_out")
            nc.sync.dma_start(out=ccA_in, in_=xs[:, :, LC:LC + 3])
            nc.gpsimd.collective_compute(
                "AllGather", OP.bypass, replica_groups=GROUPS,
                ins=[ccA_in.opt()], outs=[ccA_out.opt()])
            gA = sml.tile([128, NCHUNK, DT6, 3], f32, tag="gA")
            for j in range(NCHUNK):
                nc.sync.dma_start(out=gA[:, j], in_=ccA_out[j])
            hh = sml.tile([128, DT6, 3], f32, tag="hh")
            nc.vector.tensor_scalar(out=hh, in0=gA[:, 0], scalar1=selA_t[:, 0:1],
                                    scalar2=0.0, op0=OP.mult, op1=OP.add)
            for j in range(1, NCHUNK):
                nc.vector.scalar_tensor_tensor(
                    out=hh, in0=gA[:, j], scalar=selA_t[:, j:j + 1], in1=hh,
                    op0=OP.mult, op1=OP.add)
            nc.vector.tensor_copy(out=xs[:, :, 0:3], in_=hh)

            # conv + silu cluster
            for t in range(DT6):
                xc = scr.tile([128, LC], f32, tag="s")
                nc.vector.tensor_scalar(
                    out=xc, in0=xs[:, t, 0:LC],
                    scalar1=vdi[:, t, VDI_CW:VDI_CW + 1],
                    scalar2=vdi[:, t, VDI_CB:VDI_CB + 1],
                    op0=OP.mult, op1=OP.add)
                for k in range(1, K):
                    nc.vector.scalar_tensor_tensor(
                        out=xc, in0=xs[:, t, k:k + LC],
                        scalar=vdi[:, t, VDI_CW + k:VDI_CW + k + 1], in1=xc,
                        op0=OP.mult, op1=OP.add)
                nc.scalar.activation(out=u[:, t, :], in_=xc, func=AF.Silu)
            for t in range(DT6):
                nc.scalar.activation(out=zs[:, t, :], in_=zs[:, t, :], func=AF.Silu)

            # x_proj with 32-aligned output groups: dtl [0:24), B rows at
            # base 0 and C rows at base 32 of a second PSUM tile
            psD = psA.tile([56, LC], f32, tag="sm")
            psBC = psA.tile([56, LC], f32, tag="sm")
            psC = psA.tile([56, LC], f32, tag="sm")
            for t in range(DT6):
                nc.tensor.matmul(out=psD[0:DR, :], lhsT=w_xp[:, t, 0:DR],
                                 rhs=u[:, t, :], start=(t == 0),
                                 stop=(t == DT6 - 1))
            for t in range(DT6):
                nc.tensor.matmul(out=psBC[0:DS, :], lhsT=w_xp[:, t, DR:DR + DS],
                                 rhs=u[:, t, :], start=(t == 0),
                                 stop=(t == DT6 - 1))
            for t in range(DT6):
                nc.tensor.matmul(out=psC[0:DS, :],
                                 lhsT=w_xp[:, t, DR + DS:56],
                                 rhs=u[:, t, :], start=(t == 0),
                                 stop=(t == DT6 - 1))
            nc.scalar.copy(out=dbl[0:DR, :], in_=psD[0:DR, :])
            bcbB = wk.tile([DS, LC], bf16, tag="bcbB")
            nc.scalar.copy(out=bcbB, in_=psBC[0:DS, :])
            bcbC = wk.tile([DS, LC], bf16, tag="bcbC")
            nc.scalar.copy(out=bcbC, in_=psC[0:DS, :])
            bcd = dram.tile([2 * DS, LC], bf16, tag="bcd")
            nc.sync.dma_start(out=bcd[0:DS, :], in_=bcbB)
            nc.sync.dma_start(out=bcd[DS:2 * DS, :], in_=bcbC)

            # dt = softplus(dtl @ dt_w + dt_b) = ln(1 + exp(.)); P = exp(A*Sdt)
            pall = sml.tile([128, DT6, DS], f32, tag="pall")
            for t in range(DT6):
                psd = psB.tile([128, LC], f32, tag="mm")
                nc.tensor.matmul(out=psd, lhsT=w_dt[:, t, :], rhs=dbl[0:DR, :],
                                 start=True, stop=True)
                ex = scr.tile([128, LC], f32, tag="s")
                nc.scalar.activation(out=ex, in_=psd, func=AF.Exp,
                                     bias=vdi[:, t, VDI_DTB:VDI_DTB + 1])
                nc.scalar.activation(out=dtt[:, t, :], in_=ex, func=AF.Ln,
                                     bias=ones_f[:, 0:1])
                nc.vector.tensor_mul(dtu[:, t, :], dtt[:, t, :],
                                     u[:, t, :].bitcast(f32))
                sdt = sml.tile([128, 1], f32, tag="sdt")
                nc.vector.tensor_reduce(out=sdt, in_=dtt[:, t, :],
                                        axis=mybir.AxisListType.X, op=OP.add)
                parg = sml.tile([128, DS], f32, tag="parg")
                nc.vector.tensor_scalar_mul(out=parg, in0=aneg[:, t, :],
                                            scalar1=sdt[:, 0:1])
                nc.scalar.activation(out=pall[:, t, :], in_=parg, func=AF.Exp)

            # scan phase in n-groups; per-group carry exchange + fixup + y
            for gi, grp in enumerate(NGRPS):
                g = len(grp)
                Hb_f = hbp.tile([128, DT6, 6, LC], bf16, tag="Hb")
                CP_f = cpp.tile([128, DT6, 6, LC], bf16, tag="CP")
                Bb_f = bcp.tile([128, 6, LC], bf16, tag="Bb")
                Cb_f = bcp.tile([128, 6, LC], bf16, tag="Cb")
                Hb = Hb_f[:, :, 0:g, :]
                CPt = CP_f[:, :, 0:g, :]
                Bbs = Bb_f[:, 0:g, :]
                Cbs = Cb_f[:, 0:g, :]
                for j, n in enumerate(grp):
                    nc.sync.dma_start(
                        out=Bbs[:, j, :],
                        in_=bass.AP(tensor=bcd.tensor, offset=bcd.offset + n * LC,
                                    ap=[[0, 128], [1, LC]]))
                    nc.sync.dma_start(
                        out=Cbs[:, j, :],
                        in_=bass.AP(tensor=bcd.tensor,
                                    offset=bcd.offset + (DS + n) * LC,
                                    ap=[[0, 128], [1, LC]]))
                for t in range(DT6):
                    for j, n in enumerate(grp):
                        dA = scr.tile([128, LC], f32, tag="s")
                        nc.scalar.activation(out=dA, in_=dtt[:, t, :],
                                             func=AF.Exp,
                                             scale=aneg[:, t, n:n + 1])
                        dBu = scr.tile([128, LC], f32, tag="s")
                        nc.vector.tensor_mul(dBu, dtu[:, t, :], Bbs[:, j, :])
                        nc.vector.tensor_tensor_scan(
                            out=Hb[:, t, j, :], data0=dA, data1=dBu,
                            initial=0.0, op0=OP.mult, op1=OP.add)
                        nc.vector.tensor_tensor_scan(
                            out=CPt[:, t, j, :], data0=dA, data1=zeros512,
                            initial=1.0, op0=OP.mult, op1=OP.add)
                # pack {P, h_local_final} for this n-group and exchange
                ccBs = sml.tile([128, DT6, 2, g], f32, tag="ccBs")
                for t in range(DT6):
                    nc.vector.tensor_copy(
                        out=ccBs[:, t, 0, :],
                        in_=pall[:, t, grp[0]:grp[0] + g])
                    nc.vector.tensor_copy(
                        out=ccBs[:, t, 1, :],
                        in_=Hb[:, t, :, LC - 1:LC].rearrange("p n o -> p (n o)"))
                ccB_in = dram.tile([128, DT6, 2, 6], f32, tag="ccB_in")
                ccB_out = dram.tile([NCHUNK, 128, DT6, 2, 6], f32, tag="ccB_out")
                nc.sync.dma_start(out=ccB_in[:, :, :, 0:g], in_=ccBs)
                nc.gpsimd.collective_compute(
                    "AllGather", OP.bypass, replica_groups=GROUPS,
                    ins=[ccB_in.opt()], outs=[ccB_out.opt()])
                gB = sml.tile([128, NCHUNK, DT6, 2, 6], f32, tag="gB")
                for jj in range(NCHUNK):
                    nc.sync.dma_start(out=gB[:, jj], in_=ccB_out[jj])
                # combine prefix states over chunks, batched over (t, n)
                s1 = gB[:, 0, :, 1, 0:g]
                tmp = sml.tile([128, DT6, 6], f32, tag="cb_tmp")
                s2 = sml.tile([128, DT6, 6], f32, tag="cb_s2")
                nc.vector.tensor_mul(tmp[:, :, 0:g], gB[:, 1, :, 0, 0:g], s1)
                nc.vector.tensor_add(s2[:, :, 0:g], tmp[:, :, 0:g],
                                     gB[:, 1, :, 1, 0:g])
                tmp2 = sml.tile([128, DT6, 6], f32, tag="cb_tmp2")
                s3 = sml.tile([128, DT6, 6], f32, tag="cb_s3")
                nc.vector.tensor_mul(tmp2[:, :, 0:g], gB[:, 2, :, 0, 0:g],
                                     s2[:, :, 0:g])
                nc.vector.tensor_add(s3[:, :, 0:g], tmp2[:, :, 0:g],
                                     gB[:, 2, :, 1, 0:g])
                hin = sml.tile([128, DT6, 6], f32, tag="hin")
                nc.vector.tensor_scalar(out=hin[:, :, 0:g], in0=s1,
                                        scalar1=selB_t[:, 0:1], scalar2=0.0,
                                        op0=OP.mult, op1=OP.add)
                nc.vector.scalar_tensor_tensor(out=hin[:, :, 0:g],
                                               in0=s2[:, :, 0:g],
                                               scalar=selB_t[:, 1:2],
                                               in1=hin[:, :, 0:g],
                                               op0=OP.mult, op1=OP.add)
                nc.vector.scalar_tensor_tensor(out=hin[:, :, 0:g],
                                               in0=s3[:, :, 0:g],
                                               scalar=selB_t[:, 2:3],
                                               in1=hin[:, :, 0:g],
                                               op0=OP.mult, op1=OP.add)
                # fixup h += CP*h_in, apply C, reduce over group -> y partial
                for t in range(DT6):
                    for j, n in enumerate(grp):
                        nc.vector.scalar_tensor_tensor(
                            out=Hb[:, t, j, :], in0=CPt[:, t, j, :],
                            scalar=hin[:, t, j:j + 1], in1=Hb[:, t, j, :],
                            op0=OP.mult, op1=OP.add)
                        nc.gpsimd.tensor_mul(Hb[:, t, j, :], Hb[:, t, j, :],
                                             Cbs[:, j, :])
                    yv = Hb[:, t].rearrange("p n l -> p l n")
                    y_t = yt[:, t, :]
                    if gi == 0:
                        nc.vector.tensor_reduce(out=y_t, in_=yv,
                                                axis=mybir.AxisListType.X,
                                                op=OP.add)
                    else:
                        yp = scr.tile([128, LC], f32, tag="s")
                        nc.vector.tensor_reduce(out=yp, in_=yv,
                                                axis=mybir.AxisListType.X,
                                                op=OP.add)
                        nc.vector.tensor_add(y_t, y_t.bitcast(f32), yp)

            # y = (y + D*u) * silu(z)
            for t in range(DT6):
                y_t = yt[:, t, :]
                nc.vector.scalar_tensor_tensor(
                    out=y_t, in0=u[:, t, :].bitcast(f32),
                    scalar=vdi[:, t, VDI_DP:VDI_DP + 1], in1=y_t.bitcast(f32),
                    op0=OP.mult, op1=OP.add)
                nc.vector.tensor_mul(y_t, y_t.bitcast(f32), zs[:, t, :])

            # out_proj + residual
            for mt in range(FT):
                w_out = prm.tile([128, DT6, 128], f32r, tag="w_out")
                nc.sync.dma_start(out=w_out,
                                  in_=din["W_out"][:, li, :, mt, :].bitcast(f32r))
                ps = psB.tile([128, LC], f32, tag="mm")
                for t in range(DT6):
                    nc.tensor.matmul(out=ps, lhsT=w_out[:, t, :],
                                     rhs=yt[:, t, :],
                                     start=(t == 0), stop=(t == DT6 - 1))
                nc.vector.tensor_add(hid[:, mt, :], hid[:, mt, :].bitcast(f32), ps)

        # ---- final norm + head -------------------------------------------
        ow = cst.tile([128, FT, LAST], f32r)
        nc.sync.dma_start(out=ow, in_=din["OW"][:, :, :].bitcast(f32r))
        ob = cst.tile([LAST, 1], f32)
        nc.sync.dma_start(out=ob, in_=din["OB"][:, :])
        xf = _transposed_ln(
            nc, pools, [hid[:, mt, :] for mt in range(FT)],
            [ev2[:, mt, EV2_NFW:EV2_NFW + 1] for mt in range(FT)],
            [ev2[:, mt, EV2_NFB:EV2_NFB + 1] for mt in range(FT)],
            out_dtype=f32r)
        ps3_t = psA.tile([56, LC], f32, tag="sm")
        ps3 = ps3_t[0:LAST, :]
        for kt in range(FT):
            nc.tensor.matmul(out=ps3, lhsT=ow[:, kt, :], rhs=xf[kt],
                             start=(kt == 0), stop=(kt == FT - 1))
        outs = statep.tile([LAST, LC], f32, tag="outs")
        nc.scalar.activation(out=outs, in_=ps3, func=AF.Identity, bias=ob[:, 0:1])
        nc.sync.dma_start(out=out_t[:, :], in_=outs)

    nc.compile()
    return nc


def _prep_host(x, point_feat, language_feat, time_embedding, c_pc_xyz, params):
    """Host-side index math + data layout. Returns (in_maps, inv_f, inv_b)."""
    code_f = _morton_np(c_pc_xyz, (0, 1, 2))
    code_b = _morton_np(c_pc_xyz, (2, 1, 0))
    ord_f = np.argsort(code_f, axis=1, kind="stable")
    ord_b = np.argsort(code_b, axis=1, kind="stable")
    inv_f = np.argsort(ord_f, axis=1, kind="stable")
    inv_b = np.argsort(ord_b, axis=1, kind="stable")

    p = {k: np.asarray(v, dtype=np.float32) for k, v in params.items()}

    def pack_cols(*cols):
        return np.stack(cols, axis=-1).astype(np.float32)

    # shared param packs
    ew1 = np.zeros((INCH_PAD, 128), np.float32)
    ew1[:INCH] = p["emb_w1"]
    EW1 = ew1.reshape(13, 128, 128).transpose(1, 0, 2).copy()
    EV1 = pack_cols(p["emb_b1"], p["emb_ln1_w"], p["emb_ln1_b"])
    EW2 = p["emb_w2"].reshape(128, FT, 128).copy()
    PW1 = p["pos_w1"].copy()
    PB1 = p["pos_b1"].reshape(128, 1).copy()
    PW2 = p["pos_w2"].reshape(128, FT, 128).copy()
    ev2f = pack_cols(p["emb_b2"], p["emb_ln2_w"], p["emb_ln2_b"], p["pos_b2"],
                     p["norm_f_w"], p["norm_f_b"])          # [384, 6]
    EV2 = ev2f.reshape(FT, 128, EV2_W).transpose(1, 0, 2).copy()
    W_in = p["in_proj"].reshape(DEPTH, FT, 128, 12, 128).transpose(2, 0, 1, 3, 4).copy()
    W_out = p["out_proj"].reshape(DEPTH, DT6, 128, FT, 128).transpose(2, 0, 1, 3, 4).copy()
    W_xp = p["x_proj"].reshape(DEPTH, DT6, 128, 56).transpose(2, 0, 1, 3).copy()
    W_dt = p["dt_w"].reshape(DEPTH, DR, DT6, 128).transpose(1, 0, 2, 3).copy()
    VD = np.stack([p["ln_w"], p["ln_b"]], axis=-1)           # [12, 384, 2]
    VD = VD.reshape(DEPTH, FT, 128, 2).transpose(2, 0, 1, 3).copy()
    vdi = np.concatenate([
        p["conv_b"][..., None], p["dt_b"][..., None], p["D_p"][..., None],
        p["conv_w"], p["A_log"]], axis=-1)                   # [12, 768, 23]
    VDI = vdi.reshape(DEPTH, DT6, 128, VDI_W).transpose(2, 0, 1, 3).copy()
    OB = p["out_b"].reshape(LAST, 1).copy()
    ONESC = np.ones((128, 1), np.float32)
    ONESR = np.ones((1, 128), np.float32)

    shared = dict(EW1=EW1, EV1=EV1, EW2=EW2, PW1=PW1, PB1=PB1, PW2=PW2,
                  EV2=EV2, W_in=W_in, W_out=W_out, W_xp=W_xp, W_dt=W_dt,
                  VD=VD, VDI=VDI, ONESC=ONESC, ONESR=ONESR)

    ow_halves = [p["out_w"][:D], p["out_w"][D:]]
    osv_halves = [pack_cols(p["os_g1"], p["os_b1"]),
                  pack_cols(p["os_g2"], p["os_b2"])]

    lf = np.asarray(language_feat, np.float32)
    te = np.asarray(time_embedding, np.float32)
    x = np.asarray(x, np.float32)
    pf = np.asarray(point_feat, np.float32)
    xyz = np.asarray(c_pc_xyz, np.float32)

    in_maps = []
    for core in range(NCORES):
        b, c = core // NCHUNK, core % NCHUNK
        half = 0 if c < 2 else 1
        seq = ord_f[b] if half == 0 else ord_b[b]
        tok = seq[(c % 2) * LC:(c % 2 + 1) * LC]
        fus = np.zeros((INCH_PAD, LC), np.float32)
        fus[0:CONTACT] = x[b, tok].T
        fus[CONTACT:CONTACT + TF] = lf[b, 0][:, None]
        fus[CONTACT + TF:CONTACT + TF + TE] = te[b, 0][:, None]
        fus[CONTACT + TF + TE:CONTACT + TF + TE + PF] = pf[b, tok].T
        fus[CONTACT + TF + TE + PF:INCH] = xyz[b, tok].T
        fusionT = fus.reshape(13, 128, LC).transpose(1, 0, 2).copy()
        xyzT = xyz[b, tok].T.copy()

        selA = np.zeros((128, NCHUNK), np.float32)
        if c > 0:
            selA[:, c - 1] = 1.0
        selB = np.zeros((128, NCHUNK - 1), np.float32)
        if c > 0:
            selB[:, c - 1] = 1.0
        osvh = osv_halves[half]
        OSV = osvh.reshape(FT, 128, 2).transpose(1, 0, 2).copy()
        OW = ow_halves[half].reshape(FT, 128, LAST).transpose(1, 0, 2).copy()
        m = dict(shared)
        # out_b is added once per token pair: only by forward-half cores
        m.update(fusionT=fusionT, xyzT=xyzT, OSV=OSV, OW=OW,
                 OB=(OB if half == 0 else np.zeros_like(OB)),
                 selA=selA, selB=selB)
        in_maps.append(m)
    return in_maps, inv_f, inv_b


def kernel(x, point_feat, language_feat, time_embedding, c_pc_xyz, params):
    in_maps, inv_f, inv_b = _prep_host(x, point_feat, language_feat,
                                       time_embedding, c_pc_xyz, params)
    if "nc" not in _CACHE:
        _CACHE["nc"] = _build_nc()
    nc = _CACHE["nc"]
    trace = bool(int(os.environ.get("KERNEL_TRACE", "0")))
    res = run_bass_kernel_spmd(nc, in_maps, core_ids=list(range(NCORES)),
                               trace=trace)
    if trace and res.exec_time_ns is not None:
        print(f"HW exec time: {res.exec_time_ns} ns")
        _CACHE["exec_time_ns"] = res.exec_time_ns
    v = np.zeros((BS, LAST, L), np.float32)
    for core in range(NCORES):
        b, c = core // NCHUNK, core % NCHUNK
        v[b, :, c * LC:(c + 1) * LC] = res.results[core]["out"]
    out = np.zeros((BS, N, LAST), np.float32)
    for b in range(BS):
        out[b] = (v[b][:, inv_f[b]] + v[b][:, N + inv_b[b]]).T
    return out
